# revision 1
# baseline (speedup 1.0000x reference)
"""GPT language model forward pass on 8 Trainium2 NeuronCores.

Sharding: sequence-parallel transformer with zigzag 128-token chunks
(core c of batch-group owns chunks {c%4, 7-c%4} of its batch -> balanced
causal attention), per-layer K/V AllGather within each batch's 4-core
group, final AllGather of the normalized activations, vocab-sharded tied
LM head (4000 vocab columns per core).

All matmul operands are fp16 (fp32 PSUM accumulation); residual stream,
layernorm statistics and softmax accumulation stay fp32.
"""
import numpy as np

import concourse.bass as bass
import concourse.mybir as mybir
import concourse.tile as tile
from concourse import bacc
from concourse import bass_utils
from concourse.masks import make_identity, make_upper_triangular

F32 = mybir.dt.float32
F16 = mybir.dt.float16
AF = mybir.ActivationFunctionType

C = 1024
H = 16
D = 64
L = 8
FF = 4096
VOC = 32000
B = 2
T = 1024
NCORE = 8
VS = VOC // NCORE          # 4000
TPC = 256                  # tokens per core (2 chunks of 128)
KV_K_ELEMS = C * TPC       # 262144 (K^T contribution, fp16)
VAUG_W = H * (D + 1)       # 1040 (V plus per-head ones column)
KV_V_ELEMS = TPC * VAUG_W  # 266240
KV_ELEMS = KV_K_ELEMS + KV_V_ELEMS
X_ELEMS = C * TPC
NEG = -1.0e9
ATT_SCALE = 0.125


def _ln_stats(nc, pool, xt):
    """Return (negmu, rstd) [128,1] f32 tiles for token-major xt [128, C]."""
    stats = pool.tile([128, 2, 6], F32, tag="lnstats")
    xv = xt.rearrange("p (a c) -> p a c", a=2)
    for sg in range(2):
        nc.vector.bn_stats(out=stats[:, sg, :], in_=xv[:, sg, :])
    mv = pool.tile([128, 2], F32, tag="lnmv")
    nc.vector.bn_aggr(out=mv, in_=stats)
    negmu = pool.tile([128, 1], F32, tag="lnnegmu")
    nc.vector.tensor_scalar_mul(out=negmu, in0=mv[:, 0:1], scalar1=-1.0)
    return mv, negmu


DBG_LAYERS = L
DBG_LMHEAD = True
DBG_KVGROUP = 4
DBG_ATTN = True
DBG_WO = True
DBG_FFN = True
DBG_AG = True


def _build_program():
    nc = bacc.Bacc("TRN2", target_bir_lowering=False, debug=False,
                   enable_asserts=True, num_devices=NCORE)

    # ---- inputs (per-core maps share most arrays) ----
    x0 = nc.dram_tensor("x0", [TPC, C], F32, kind="ExternalInput")
    kmask = nc.dram_tensor("kmask", [16], F32, kind="ExternalInput")
    wq = nc.dram_tensor("wq", [L, C, C], F16, kind="ExternalInput")
    wk = nc.dram_tensor("wk", [L, C, C], F16, kind="ExternalInput")
    wv = nc.dram_tensor("wv", [L, C, C], F16, kind="ExternalInput")
    wo = nc.dram_tensor("wo", [L, C, C], F16, kind="ExternalInput")
    w1b = nc.dram_tensor("w1b", [L, FF // 128, 8, 128, 128], F16,
                         kind="ExternalInput")
    w2 = nc.dram_tensor("w2", [L, FF, C], F16, kind="ExternalInput")
    bo16 = nc.dram_tensor("bo16", [L, C], F16, kind="ExternalInput")
    b2_16 = nc.dram_tensor("b2_16", [L, C], F16, kind="ExternalInput")
    b1f = nc.dram_tensor("b1f", [L, FF], F32, kind="ExternalInput")
    ln1s = nc.dram_tensor("ln1s", [L, C], F32, kind="ExternalInput")
    ln1b = nc.dram_tensor("ln1b", [L, C], F32, kind="ExternalInput")
    ln2s = nc.dram_tensor("ln2s", [L, C], F32, kind="ExternalInput")
    ln2b = nc.dram_tensor("ln2b", [L, C], F32, kind="ExternalInput")
    lnfs = nc.dram_tensor("lnfs", [C], F32, kind="ExternalInput")
    lnfb = nc.dram_tensor("lnfb", [C], F32, kind="ExternalInput")
    embT = nc.dram_tensor("embT", [C, VS], F16, kind="ExternalInput")
    logits = nc.dram_tensor("logits", [B * T, VS], F32, kind="ExternalOutput")

    with tile.TileContext(nc) as tc:
        _body(nc, tc, locals())
    nc.compile()
    return nc


def _body(nc, tc, t):
    from contextlib import ExitStack
    ctx = ExitStack()
    with ctx:
        per = ctx.enter_context(tc.tile_pool(name="per", bufs=1))
        xpool = ctx.enter_context(tc.tile_pool(name="xpool", bufs=2))
        hpool = ctx.enter_context(tc.tile_pool(name="hpool", bufs=2))
        tmp = ctx.enter_context(tc.tile_pool(name="tmp", bufs=2))
        small = ctx.enter_context(tc.tile_pool(name="small", bufs=3))
        wsl = ctx.enter_context(tc.tile_pool(name="wsl", bufs=10))
        w1p = ctx.enter_context(tc.tile_pool(name="w1p", bufs=3))
        apool = ctx.enter_context(tc.tile_pool(name="apool", bufs=3))
        epool = ctx.enter_context(tc.tile_pool(name="epool", bufs=4))
        evp = ctx.enter_context(tc.tile_pool(name="evp", bufs=2))
        dram = ctx.enter_context(tc.tile_pool(name="dram", bufs=2,
                                              space="DRAM"))
        pbig = ctx.enter_context(tc.tile_pool(name="pbig", bufs=2,
                                              space="PSUM"))
        pmed = ctx.enter_context(tc.tile_pool(name="pmed", bufs=2,
                                              space="PSUM"))
        pacc = ctx.enter_context(tc.tile_pool(name="pacc", bufs=1,
                                              space="PSUM"))

        # ---- persistent constants ----
        ident = per.tile([128, 128], F32)
        make_identity(nc, ident)
        tril = per.tile([128, 128], F16)  # [k,q] keep k<=q
        make_upper_triangular(nc, tril, val=1.0, diag=True)
        kmask_sb = per.tile([128, 16], F32)
        nc.gpsimd.dma_start(
            out=kmask_sb,
            in_=bass.AP(tensor=t["kmask"], offset=0, ap=[[0, 128], [1, 16]]))
        eps_t = per.tile([128, 1], F32)
        nc.vector.memset(eps_t, 1e-5)
        ones1 = per.tile([1, 128], F16)
        nc.vector.memset(ones1, 1.0)

        QT = per.tile([128, 8 * TPC], F16)       # Q^T fm, col=ct*256+tok
        KT_own = per.tile([128, 8 * TPC], F16)
        V_own = per.tile([128, 2, H, D + 1], F16)
        nc.vector.memset(V_own[:, :, :, D:D + 1], 1.0)
        KT_g = per.tile([128, 8, 8, 128], F16)   # [p, ct, j, tok]
        V_g = per.tile([128, 8, H, D + 1], F16)  # per j-chunk aug V
        # V_g needs per-j [128, 1040] -> [128, 8, 16, 65]
        nc.vector.memset(V_g[:, :, :, D:D + 1], 1.0)
        o_sb = per.tile([128, 2, C], F32)        # attention out, token-major
        oT = per.tile([128, 8 * TPC], F16)       # o^T fm
        xfT = per.tile([128, 8 * TPC], F16)      # final-LN x^T fm
        XT = per.tile([128, 8, 8, TPC], F16)     # gathered X^T [p, ct, r, tok]

        x_cur = []
        for ci in range(2):
            xt = xpool.tile([128, C], F32, tag=f"x{ci}")
            nc.sync.dma_start(out=xt, in_=t["x0"][ci * 128:(ci + 1) * 128, :])
            x_cur.append(xt)

        def ln_transpose(xt_pair, s_dram, b_dram, l, dstT, cast16=True):
            """LayerNorm (token-major) + transpose + scale/bias in fm."""
            scol = small.tile([128, 8], F32, tag="scol")
            bcol = small.tile([128, 8], F32, tag="bcol")
            src_s = s_dram[l] if l is not None else s_dram
            src_b = b_dram[l] if l is not None else b_dram
            nc.sync.dma_start(out=scol,
                              in_=src_s.rearrange("(a b) -> b a", b=128))
            nc.sync.dma_start(out=bcol,
                              in_=src_b.rearrange("(a b) -> b a", b=128))
            for ci in range(2):
                xt = xt_pair[ci]
                mv, negmu = _ln_stats(nc, small, xt)
                sq = small.tile([128, 1], F32, tag="lnsq")
                nc.scalar.activation(out=sq, in_=mv[:, 1:2], func=AF.Sqrt,
                                     bias=eps_t, scale=1.0)
                rstd = small.tile([128, 1], F32, tag="lnrstd")
                nc.vector.reciprocal(out=rstd, in_=sq)
                xn = tmp.tile([128, C], F32, tag="xn")
                nc.vector.tensor_scalar(out=xn, in0=xt, scalar1=negmu,
                                        scalar2=rstd,
                                        op0=mybir.AluOpType.add,
                                        op1=mybir.AluOpType.mult)
                for ct in range(8):
                    tp = pmed.tile([128, 128], F32, tag="med")
                    nc.tensor.transpose(tp, xn[:, ct * 128:(ct + 1) * 128],
                                        ident)
                    nc.vector.tensor_scalar(
                        out=dstT[:, ct * 256 + ci * 128:
                                 ct * 256 + ci * 128 + 128],
                        in0=tp, scalar1=scol[:, ct:ct + 1],
                        scalar2=bcol[:, ct:ct + 1],
                        op0=mybir.AluOpType.mult, op1=mybir.AluOpType.add)

        def proj_fm(w_dram, l, dstT):
            """dstT[feat, tok] (fp16) = W^T @ h^T, W [C,C] fm-major."""
            slabs = []
            for ct in range(8):
                sl = wsl.tile([128, C], F16, tag="wsl")
                nc.sync.dma_start(out=sl,
                                  in_=w_dram[l, ct * 128:(ct + 1) * 128, :])
                slabs.append(sl)
            for grp in range(8):
                ps = pmed.tile([128, 256], F32, tag="med")
                for ct in range(8):
                    nc.tensor.matmul(
                        ps, slabs[ct][:, grp * 128:(grp + 1) * 128],
                        hT[:, ct * 256:(ct + 1) * 256],
                        start=(ct == 0), stop=(ct == 7))
                nc.vector.tensor_copy(
                    out=dstT[:, grp * 256:(grp + 1) * 256], in_=ps)
            return slabs

        for l in range(DBG_LAYERS):
            # ---- LN1 + h^T ----
            hT = hpool.tile([128, 8 * TPC], F16, tag="hT")
            ln_transpose(x_cur, t["ln1s"], t["ln1b"], l, hT)

            # ---- K, V projections first (feed the AllGather) ----
            proj_fm(t["wk"], l, KT_own)
            vslabs = []
            for ct in range(8):
                sl = wsl.tile([128, C], F16, tag="wsl")
                nc.sync.dma_start(out=sl,
                                  in_=t["wv"][l, ct * 128:(ct + 1) * 128, :])
                vslabs.append(sl)
            for ci in range(2):
                ps = pbig.tile([128, C], F32, tag="big")
                for half in range(2):
                    for ct in range(8):
                        nc.tensor.matmul(
                            ps[:, half * 512:(half + 1) * 512],
                            hT[:, ct * 256 + ci * 128:ct * 256 + ci * 128 + 128],
                            vslabs[ct][:, half * 512:(half + 1) * 512],
                            start=(ct == 0), stop=(ct == 7))
                for half in range(2):
                    nc.vector.tensor_copy(
                        out=V_own[:, ci, half * 8:(half + 1) * 8, 0:D],
                        in_=ps[:, half * 512:(half + 1) * 512]
                        .rearrange("p (h d) -> p h d", h=8))

            # ---- launch K/V AllGather (per-batch groups of 4) ----
            ag_in = dram.tile([KV_ELEMS], F16, tag="agin")
            ag_out = dram.tile([DBG_KVGROUP * KV_ELEMS], F16, tag="agout")
            nc.sync.dma_start(
                out=ag_in[0:KV_K_ELEMS].rearrange("(a p c) -> p a c",
                                                  a=8, p=128),
                in_=KT_own[:].rearrange("p (a c) -> p a c", a=8))
            nc.sync.dma_start(
                out=ag_in[KV_K_ELEMS:KV_ELEMS].rearrange("(a p c) -> p a c",
                                                         a=2, p=128),
                in_=V_own[:].rearrange("p a h d -> p a (h d)"))
            kv_groups = ([[0, 1, 2, 3], [4, 5, 6, 7]] if DBG_KVGROUP == 4
                         else [list(range(8))])
            if DBG_AG:
                nc.gpsimd.collective_compute(
                    "AllGather", mybir.AluOpType.bypass,
                    replica_groups=kv_groups,
                    ins=[ag_in[:].opt()], outs=[ag_out[:].opt()])
            else:
                nc.sync.dma_start(out=ag_out[0:KV_ELEMS], in_=ag_in[:])

            # ---- Q projection (overlaps with AllGather) ----
            proj_fm(t["wq"], l, QT)

            # ---- load gathered K/V into global-chunk order ----
            for r in range(4):
                base = r * KV_ELEMS
                kv = ag_out[base:base + KV_K_ELEMS].rearrange(
                    "(a p c) -> a p c", a=8, p=128)
                vv = ag_out[base + KV_K_ELEMS:base + KV_ELEMS].rearrange(
                    "(a p c) -> a p c", a=2, p=128)
                for sub in range(2):
                    j = r if sub == 0 else 7 - r
                    nc.sync.dma_start(
                        out=KT_g[:, :, j, :],
                        in_=kv[:, :, sub * 128:(sub + 1) * 128]
                        .rearrange("a p c -> p a c"))
                    nc.sync.dma_start(
                        out=V_g[:, j, :, :],
                        in_=vv[sub, :, :].rearrange("p (h d) -> p h d", h=H))

            # ---- attention ----
            for h in range(H if DBG_ATTN else 0):
                po = (h % 2) * 64
                cth = h // 2
                o_aug = [pacc.tile([128, D + 1], F32, tag=f"oa{qc}",
                                   name=f"oaug{qc}_{l}_{h}")
                         for qc in range(2)]
                # local units: (qc, kc, masked)
                for (qc, kc, msk, first) in ((0, 0, True, True),
                                             (1, 1, True, True),
                                             (1, 0, False, False)):
                    ps = pmed.tile([128, 128], F32, tag="med")
                    nc.tensor.matmul(
                        ps,
                        KT_own[po:po + 64,
                               cth * 256 + kc * 128:cth * 256 + kc * 128 + 128],
                        QT[po:po + 64,
                           cth * 256 + qc * 128:cth * 256 + qc * 128 + 128],
                        start=True, stop=True)
                    e = epool.tile([128, 128], F16, tag="e")
                    nc.scalar.activation(out=e, in_=ps, func=AF.Exp,
                                         scale=ATT_SCALE)
                    if msk:
                        nc.vector.tensor_mul(out=e, in0=e, in1=tril)
                    nc.tensor.matmul(o_aug[qc], e,
                                     V_own[:, kc, h, :],
                                     start=first, stop=False)
                # remote units vs all 8 global chunks (masks from data)
                for j in range(8):
                    ps = pmed.tile([128, 256], F32, tag="med")
                    nc.tensor.matmul(
                        ps,
                        KT_g[po:po + 64, cth, j, :],
                        QT[po:po + 64, cth * 256:(cth + 1) * 256],
                        start=True, stop=True)
                    for qc in range(2):
                        e = epool.tile([128, 128], F16, tag="e")
                        nc.scalar.activation(
                            out=e, in_=ps[:, qc * 128:(qc + 1) * 128],
                            func=AF.Exp, scale=ATT_SCALE,
                            bias=kmask_sb[:, 8 * qc + j:8 * qc + j + 1])
                        nc.tensor.matmul(o_aug[qc], e, V_g[:, j, h, :],
                                         start=False, stop=(j == 7))
                for qc in range(2):
                    rec = small.tile([128, 1], F32, tag="rec")
                    nc.vector.reciprocal(out=rec, in_=o_aug[qc][:, D:D + 1])
                    nc.vector.tensor_scalar_mul(
                        out=o_sb[:, qc, h * D:(h + 1) * D],
                        in0=o_aug[qc][:, 0:D], scalar1=rec)

            # ---- output projection + residual ----
            for ci in range(2 if DBG_WO else 0):
                for ct in range(8):
                    tp = pmed.tile([128, 128], F32, tag="med")
                    nc.tensor.transpose(
                        tp, o_sb[:, ci, ct * 128:(ct + 1) * 128], ident)
                    nc.vector.tensor_copy(
                        out=oT[:, ct * 256 + ci * 128:ct * 256 + ci * 128 + 128],
                        in_=tp)
            woslabs = []
            for ct in range(8):
                sl = wsl.tile([128, C], F16, tag="wsl")
                nc.sync.dma_start(out=sl,
                                  in_=t["wo"][l, ct * 128:(ct + 1) * 128, :])
                woslabs.append(sl)
            bo_sb = small.tile([1, C], F16, tag="borow")
            nc.sync.dma_start(out=bo_sb, in_=t["bo16"][l:l + 1, :])
            x_new = []
            for ci in range(2):
                ps = pbig.tile([128, C], F32, tag="big")
                for half in range(2):
                    nc.tensor.matmul(ps[:, half * 512:(half + 1) * 512],
                                     ones1,
                                     bo_sb[:, half * 512:(half + 1) * 512],
                                     start=True, stop=False)
                    for ct in range(8):
                        nc.tensor.matmul(
                            ps[:, half * 512:(half + 1) * 512],
                            oT[:, ct * 256 + ci * 128:ct * 256 + ci * 128 + 128],
                            woslabs[ct][:, half * 512:(half + 1) * 512],
                            start=False, stop=(ct == 7))
                xt = xpool.tile([128, C], F32, tag=f"x{ci}")
                nc.vector.tensor_tensor(out=xt, in0=ps, in1=x_cur[ci],
                                        op=mybir.AluOpType.add)
                x_new.append(xt)
            if x_new:
                x_cur = x_new

            # ---- LN2 + FFN ----
            if not DBG_FFN:
                continue
            hT = hpool.tile([128, 8 * TPC], F16, tag="hT")
            ln_transpose(x_cur, t["ln2s"], t["ln2b"], l, hT)
            b1col = small.tile([128, FF // 128], F32, tag="b1col")
            nc.sync.dma_start(out=b1col,
                              in_=t["b1f"][l].rearrange("(a b) -> b a", b=128))
            b2_sb = small.tile([1, C], F16, tag="b2row")
            nc.sync.dma_start(out=b2_sb, in_=t["b2_16"][l:l + 1, :])
            ps_f2 = []
            for ci in range(2):
                ps = pbig.tile([128, C], F32, tag="big")
                for half in range(2):
                    nc.tensor.matmul(ps[:, half * 512:(half + 1) * 512],
                                     ones1,
                                     b2_sb[:, half * 512:(half + 1) * 512],
                                     start=True, stop=False)
                ps_f2.append(ps)
            for f in range(FF // 128):
                w1t = w1p.tile([128, 8 * 128], F16, tag="w1t")
                nc.sync.dma_start(
                    out=w1t[:].rearrange("p (a c) -> p a c", a=8),
                    in_=t["w1b"][l, f].rearrange("a p c -> p a c"))
                ps1 = pmed.tile([128, 256], F32, tag="med")
                for ct in range(8):
                    nc.tensor.matmul(ps1, w1t[:, ct * 128:(ct + 1) * 128],
                                     hT[:, ct * 256:(ct + 1) * 256],
                                     start=(ct == 0), stop=(ct == 7))
                aT = apool.tile([128, 256], F16, tag="aT")
                nc.scalar.activation(out=aT, in_=ps1, func=AF.Relu,
                                     bias=b1col[:, f:f + 1], scale=1.0)
                w2t = wsl.tile([128, C], F16, tag="wsl")
                nc.sync.dma_start(out=w2t,
                                  in_=t["w2"][l, f * 128:(f + 1) * 128, :])
                for ci in range(2):
                    for half in range(2):
                        nc.tensor.matmul(
                            ps_f2[ci][:, half * 512:(half + 1) * 512],
                            aT[:, ci * 128:(ci + 1) * 128],
                            w2t[:, half * 512:(half + 1) * 512],
                            start=False, stop=(f == FF // 128 - 1))
            x_new = []
            for ci in range(2):
                xt = xpool.tile([128, C], F32, tag=f"x{ci}")
                nc.vector.tensor_tensor(out=xt, in0=ps_f2[ci], in1=x_cur[ci],
                                        op=mybir.AluOpType.add)
                x_new.append(xt)
            x_cur = x_new

        # ---- final layernorm -> x^T fm fp16 -> AllGather all 8 cores ----
        ln_transpose(x_cur, t["lnfs"], t["lnfb"], None, xfT)
        agx_in = dram.tile([X_ELEMS], F16, tag="agxin")
        agx_out = dram.tile([NCORE * X_ELEMS], F16, tag="agxout",
                            addr_space="Shared")
        nc.sync.dma_start(
            out=agx_in[:].rearrange("(a p c) -> p a c", a=8, p=128),
            in_=xfT[:].rearrange("p (a c) -> p a c", a=8))
        nc.gpsimd.collective_compute(
            "AllGather", mybir.AluOpType.bypass,
            replica_groups=[list(range(NCORE))],
            ins=[agx_in[:].opt()], outs=[agx_out[:].opt()])
        for r in range(NCORE):
            nc.sync.dma_start(
                out=XT[:, :, r, :],
                in_=agx_out[r * X_ELEMS:(r + 1) * X_ELEMS]
                .rearrange("(a p c) -> p a c", a=8, p=128))

        # ---- LM head: logits[2048, 4000] = X^T.T @ embT ----
        for vg in range(8 if DBG_LMHEAD else 0):
            evt = evp.tile([128, 8, 500], F16, tag="evt")
            nc.sync.dma_start(
                out=evt,
                in_=t["embT"].rearrange("(a p) v -> p a v", p=128)
                [:, :, vg * 500:(vg + 1) * 500])
            for tt in range(16):
                ps = pmed.tile([128, 500], F32, tag="med")
                for ct in range(8):
                    nc.tensor.matmul(ps, XT[:, ct, tt // 2,
                                            (tt % 2) * 128:(tt % 2) * 128 + 128],
                                     evt[:, ct, :],
                                     start=(ct == 0), stop=(ct == 7))
                lstage = apool.tile([128, 500], F32, tag="lstage")
                nc.vector.tensor_copy(out=lstage, in_=ps)
                nc.sync.dma_start(
                    out=t["logits"][tt * 128:(tt + 1) * 128,
                                    vg * 500:(vg + 1) * 500],
                    in_=lstage)


_PROG = None


def _get_program():
    global _PROG
    if _PROG is None:
        _PROG = _build_program()
    return _PROG


def _host_inputs(idx, tok_emb, pos_emb, ln1_s, ln1_b, Wq, Wk, Wv, Wo, bo,
                 ln2_s, ln2_b, W1, b1, W2, b2, lnf_s, lnf_b):
    f16 = np.float16
    emb = (tok_emb[idx] + pos_emb[None, :, :]).astype(np.float32)  # (B,T,C)
    wq_t = np.ascontiguousarray(
        np.transpose(Wq, (0, 2, 1, 3)).reshape(L, C, C).astype(f16))
    wk_t = np.ascontiguousarray(
        np.transpose(Wk, (0, 2, 1, 3)).reshape(L, C, C).astype(f16))
    wv_t = np.ascontiguousarray(
        np.transpose(Wv, (0, 2, 1, 3)).reshape(L, C, C).astype(f16))
    w1bk = np.ascontiguousarray(
        W1.reshape(L, 8, 128, 32, 128).transpose(0, 3, 1, 2, 4).astype(f16))
    shared = {
        "wq": wq_t, "wk": wk_t, "wv": wv_t,
        "wo": np.ascontiguousarray(Wo.astype(f16)),
        "w1b": w1bk,
        "w2": np.ascontiguousarray(W2.astype(f16)),
        "bo16": bo.astype(f16), "b2_16": b2.astype(f16),
        "b1f": b1.astype(np.float32),
        "ln1s": ln1_s.astype(np.float32), "ln1b": ln1_b.astype(np.float32),
        "ln2s": ln2_s.astype(np.float32), "ln2b": ln2_b.astype(np.float32),
        "lnfs": lnf_s.astype(np.float32), "lnfb": lnf_b.astype(np.float32),
    }
    embT_full = np.ascontiguousarray(tok_emb.T.astype(f16))  # (C, VOC)
    in_maps = []
    for c in range(NCORE):
        b = c // 4
        c4 = c % 4
        g1, g2 = c4, 7 - c4
        x0 = np.concatenate([emb[b, g1 * 128:(g1 + 1) * 128],
                             emb[b, g2 * 128:(g2 + 1) * 128]], axis=0)
        km = np.full(16, NEG, np.float32)
        for j in range(8):
            if j < g1:
                km[j] = 0.0
            if j < g2 and j != g1:
                km[8 + j] = 0.0
        m = dict(shared)
        m["x0"] = np.ascontiguousarray(x0.astype(np.float32))
        m["kmask"] = km
        m["embT"] = np.ascontiguousarray(embT_full[:, c * VS:(c + 1) * VS])
        in_maps.append(m)
    return in_maps


def _assemble(results):
    """results: list of per-core dicts with 'logits' [2048, VS] in gathered
    token order (rank-major zigzag chunks). Returns (B, T, VOC) f32."""
    out = np.empty((B, T, VOC), np.float32)
    for c in range(NCORE):
        lg = results[c]["logits"]  # rows: rank r block 256 = chunks (g1,g2)
        vsl = slice(c * VS, (c + 1) * VS)
        for r in range(NCORE):
            rb = r // 4
            r4 = r % 4
            g1, g2 = r4, 7 - r4
            blk = lg[r * 256:(r + 1) * 256]
            out[rb, g1 * 128:(g1 + 1) * 128, vsl] = blk[0:128]
            out[rb, g2 * 128:(g2 + 1) * 128, vsl] = blk[128:256]
    return out


def kernel(**inputs):
    nc = _get_program()
    in_maps = _host_inputs(**inputs)
    res = bass_utils.run_bass_kernel_spmd(
        nc, in_maps, core_ids=list(range(NCORE)))
    return _assemble(res.results)


if __name__ == "__main__":
    import reference as R
    inp = {k: np.asarray(v) for k, v in R.setup_inputs().items()}
    out = kernel(**inp)
    exp = np.asarray(R.reference(**inp))
    err = np.abs(out - exp)
    print("absmax expected:", np.abs(exp).max())
    print("max abs err:", err.max(),
          "rel:", err.max() / np.abs(exp).max())



# revision 8
# speedup vs baseline: 1.1876x; 1.1876x over previous
"""GPT language model forward pass on 8 Trainium2 NeuronCores.

Sequence-parallel transformer: zigzag 128-token chunks (core c of a
4-core batch group owns global chunks {c%4, 7-c%4}), split K / V
AllGathers per layer (each overlapped with the following projections),
superset-static attention (qc0 vs remote chunks {0,1,2}+diag, qc1 vs
{0..6}+diag) with per-core multiplicative masks, one exp per (head,qc),
all transposes on the DMA XBAR, vocab-sharded tied LM head.

Matmul operands are fp16 (fp32 PSUM); residual stream, layernorm stats
and softmax accumulation stay fp32.
"""
import numpy as np

import concourse.bass as bass
import concourse.mybir as mybir
import concourse.tile as tile
from concourse import bacc
from concourse import bass_utils

F32 = mybir.dt.float32
F16 = mybir.dt.float16
AF = mybir.ActivationFunctionType
ALU = mybir.AluOpType

C = 1024
H = 16
D = 64
L = 8
FF = 4096
VOC = 32000
B = 2
T = 1024
NCORE = 8
VS = VOC // NCORE          # 4000
TPC = 256                  # tokens per core (2 chunks of 128)
K_ELEMS = 8 * 128 * TPC    # staged K^T elements per core (fp16)
V_ELEMS = 2 * 128 * H * (D + 1)
X_ELEMS = C * TPC
ATT_SCALE = 0.125

import os as _os
DBG_LAYERS = int(_os.environ.get("DBG_LAYERS", L))
DBG_LMHEAD = _os.environ.get("DBG_LMHEAD", "1") == "1"
DBG_ATTN = _os.environ.get("DBG_ATTN", "1") == "1"
DBG_WO = _os.environ.get("DBG_WO", "1") == "1"
DBG_FFN = _os.environ.get("DBG_FFN", "1") == "1"


def _ln_stats(nc, pool, xt):
    """Return (mv, negmu) for token-major xt [128, C]."""
    stats = pool.tile([128, 2, 6], F32, tag="lnstats")
    xv = xt.rearrange("p (a c) -> p a c", a=2)
    for sg in range(2):
        nc.vector.bn_stats(out=stats[:, sg, :], in_=xv[:, sg, :])
    mv = pool.tile([128, 2], F32, tag="lnmv")
    nc.vector.bn_aggr(out=mv, in_=stats)
    negmu = pool.tile([128, 1], F32, tag="lnnegmu")
    nc.vector.tensor_scalar_mul(out=negmu, in0=mv[:, 0:1], scalar1=-1.0)
    return mv, negmu


def _build_program():
    nc = bacc.Bacc("TRN2", target_bir_lowering=False, debug=False,
                   enable_asserts=True, num_devices=NCORE)

    x0 = nc.dram_tensor("x0", [TPC, C], F32, kind="ExternalInput")
    m0d = nc.dram_tensor("m0d", [128, 512], F16, kind="ExternalInput")
    m1d = nc.dram_tensor("m1d", [128, 1024], F16, kind="ExternalInput")
    wq = nc.dram_tensor("wq", [L, C, C], F16, kind="ExternalInput")
    wk = nc.dram_tensor("wk", [L, C, C], F16, kind="ExternalInput")
    wv = nc.dram_tensor("wv", [L, C, C], F16, kind="ExternalInput")
    wo = nc.dram_tensor("wo", [L, C, C], F16, kind="ExternalInput")
    w1b = nc.dram_tensor("w1b", [L, FF // 128, 8, 128, 128], F16,
                         kind="ExternalInput")
    w2 = nc.dram_tensor("w2", [L, FF, C], F16, kind="ExternalInput")
    bo16 = nc.dram_tensor("bo16", [L, C], F16, kind="ExternalInput")
    b2_16 = nc.dram_tensor("b2_16", [L, C], F16, kind="ExternalInput")
    b1f = nc.dram_tensor("b1f", [L, FF], F32, kind="ExternalInput")
    ln1s = nc.dram_tensor("ln1s", [L, C], F32, kind="ExternalInput")
    ln1b = nc.dram_tensor("ln1b", [L, C], F32, kind="ExternalInput")
    ln2s = nc.dram_tensor("ln2s", [L, C], F32, kind="ExternalInput")
    ln2b = nc.dram_tensor("ln2b", [L, C], F32, kind="ExternalInput")
    lnfs = nc.dram_tensor("lnfs", [C], F32, kind="ExternalInput")
    lnfb = nc.dram_tensor("lnfb", [C], F32, kind="ExternalInput")
    embT = nc.dram_tensor("embT", [C, VS], F16, kind="ExternalInput")
    logits = nc.dram_tensor("logits", [B * T, VS], F16, kind="ExternalOutput")

    with tile.TileContext(nc) as tc:
        _body(nc, tc, locals())
    nc.compile()
    return nc


def _body(nc, tc, t):
    from contextlib import ExitStack
    ctx = ExitStack()
    with ctx:
        per = ctx.enter_context(tc.tile_pool(name="per", bufs=1))
        xpool = ctx.enter_context(tc.tile_pool(name="xpool", bufs=2))
        hpool = ctx.enter_context(tc.tile_pool(name="hpool", bufs=2))
        tmp = ctx.enter_context(tc.tile_pool(name="tmp", bufs=2))
        small = ctx.enter_context(tc.tile_pool(name="small", bufs=4))
        wsl = ctx.enter_context(tc.tile_pool(name="wsl", bufs=12))
        w1p = ctx.enter_context(tc.tile_pool(name="w1p", bufs=3))
        w2p = ctx.enter_context(tc.tile_pool(name="w2p", bufs=3))
        apool = ctx.enter_context(tc.tile_pool(name="apool", bufs=3))
        epool = ctx.enter_context(tc.tile_pool(name="epool", bufs=3))
        evp = ctx.enter_context(tc.tile_pool(name="evp", bufs=2))
        dram = ctx.enter_context(tc.tile_pool(name="dram", bufs=2,
                                              space="DRAM"))
        pbig = ctx.enter_context(tc.tile_pool(name="pbig", bufs=2,
                                              space="PSUM"))
        pmed = ctx.enter_context(tc.tile_pool(name="pmed", bufs=2,
                                              space="PSUM"))
        pacc = ctx.enter_context(tc.tile_pool(name="pacc", bufs=2,
                                              space="PSUM"))

        # ---- persistent tiles ----
        QT = per.tile([128, 8 * TPC], F16)       # col = ct*256 + ci*128 + t
        KT_own = per.tile([128, 8 * TPC], F16)
        V_own = per.tile([128, 2, H, D + 1], F16)
        nc.vector.memset(V_own[:, :, :, D:D + 1], 1.0)
        KT_g = per.tile([128, 8, 4, 2, 128], F16)   # [p, ct, r, sub, tok]
        V_g = per.tile([128, 4, 2, H, D + 1], F16)  # [p, r, sub, h, d]
        m0 = per.tile([128, 512], F16)
        m1 = per.tile([128, 1024], F16)
        o16 = per.tile([128, 2, C], F16)         # attn out, token-major fp16
        oT = per.tile([128, 8 * TPC], F16)
        xfT = per.tile([128, 8 * TPC], F16)
        XT = per.tile([128, 8, 8, TPC], F16)     # [p, ct, r, tok]
        eps_t = per.tile([128, 1], F32)
        nc.vector.memset(eps_t, 1e-5)
        ones1 = per.tile([1, 128], F16)
        nc.vector.memset(ones1, 1.0)

        # warmup collective: absorbs the runtime's first-collective
        # bootstrap cost concurrently with layer-0 compute
        wseed = small.tile([1, 256], F16, tag="wseed")
        nc.vector.memset(wseed, 0.0)
        warm_in = dram.tile([256], F16, tag="warmin")
        warm_out = dram.tile([NCORE * 256], F16, tag="warmout",
                             addr_space="Shared")
        nc.sync.dma_start(
            out=warm_in[:].rearrange("(a b) -> a b", a=1), in_=wseed)
        nc.gpsimd.collective_compute(
            "AllGather", ALU.bypass, replica_groups=[list(range(NCORE))],
            ins=[warm_in[:].opt()], outs=[warm_out[:].opt()])

        nc.sync.dma_start(out=m0, in_=t["m0d"][:, :])
        nc.sync.dma_start(out=m1, in_=t["m1d"][:, :])

        x_cur = []
        for ci in range(2):
            xt = xpool.tile([128, C], F32, tag=f"x{ci}")
            nc.sync.dma_start(out=xt, in_=t["x0"][ci * 128:(ci + 1) * 128, :])
            x_cur.append(xt)

        def ln_to_fm(xt_pair, s_dram, b_dram, l, dstT):
            """LayerNorm token-major -> fp16 -> DMA-XBAR transpose into
            feature-major dstT, then per-feature scale/bias."""
            scol = small.tile([128, 8], F32, tag="scol")
            bcol = small.tile([128, 8], F32, tag="bcol")
            src_s = s_dram[l] if l is not None else s_dram
            src_b = b_dram[l] if l is not None else b_dram
            nc.sync.dma_start(out=scol,
                              in_=src_s.rearrange("(a b) -> b a", b=128))
            nc.sync.dma_start(out=bcol,
                              in_=src_b.rearrange("(a b) -> b a", b=128))
            dview = dstT[:].rearrange("p (a two t) -> p a two t", a=8, two=2)
            for ci in range(2):
                xt = xt_pair[ci]
                mv, negmu = _ln_stats(nc, small, xt)
                sq = small.tile([128, 1], F32, tag="lnsq")
                nc.scalar.activation(out=sq, in_=mv[:, 1:2], func=AF.Sqrt,
                                     bias=eps_t, scale=1.0)
                rstd = small.tile([128, 1], F32, tag="lnrstd")
                nc.vector.reciprocal(out=rstd, in_=sq)
                xn16 = tmp.tile([128, C], F16, tag="xn")
                nc.vector.tensor_scalar(out=xn16, in0=xt, scalar1=negmu,
                                        scalar2=rstd,
                                        op0=ALU.add, op1=ALU.mult)
                nc.scalar.dma_start_transpose(out=dview[:, :, ci, :],
                                              in_=xn16[:])
            for ct in range(8):
                sl = dstT[:, ct * 256:(ct + 1) * 256]
                nc.vector.tensor_scalar(out=sl, in0=sl,
                                        scalar1=scol[:, ct:ct + 1],
                                        scalar2=bcol[:, ct:ct + 1],
                                        op0=ALU.mult, op1=ALU.add)

        def load_slabs(w_dram, l):
            slabs = []
            for ct in range(8):
                sl = wsl.tile([128, C], F16, tag="wsl")
                nc.sync.dma_start(out=sl,
                                  in_=w_dram[l, ct * 128:(ct + 1) * 128, :])
                slabs.append(sl)
            return slabs

        def proj_fm(slabs, hT, dstT):
            """dstT[feat, tok] fp16 = W^T @ h^T."""
            for grp in range(8):
                ps = pmed.tile([128, 512], F32, tag="med")
                for ct in range(8):
                    nc.tensor.matmul(
                        ps[:, 0:256], slabs[ct][:, grp * 128:(grp + 1) * 128],
                        hT[:, ct * 256:(ct + 1) * 256],
                        start=(ct == 0), stop=(ct == 7))
                nc.vector.tensor_copy(
                    out=dstT[:, grp * 256:(grp + 1) * 256], in_=ps[:, 0:256])

        # chunk j -> (rank, sub) in the gathered buffers
        def rs(j):
            return (j, 0) if j < 4 else (7 - j, 1)

        for l in range(DBG_LAYERS):
            # ---- LN1 ----
            hT = hpool.tile([128, 8 * TPC], F16, tag="hT")
            ln_to_fm(x_cur, t["ln1s"], t["ln1b"], l, hT)

            # ---- K projection -> stage -> AllGather K ----
            kslabs = load_slabs(t["wk"], l)
            proj_fm(kslabs, hT, KT_own)
            agk_in = dram.tile([K_ELEMS], F16, tag="agkin")
            agk_out = dram.tile([4 * K_ELEMS], F16, tag="agkout")
            nc.sync.dma_start(
                out=agk_in[:].rearrange("(a p c) -> p a c", a=8, p=128),
                in_=KT_own[:].rearrange("p (a c) -> p a c", a=8))
            nc.gpsimd.collective_compute(
                "AllGather", ALU.bypass,
                replica_groups=[[0, 1, 2, 3], [4, 5, 6, 7]],
                ins=[agk_in[:].opt()], outs=[agk_out[:].opt()])

            # ---- V projection -> stage -> AllGather V ----
            vslabs = load_slabs(t["wv"], l)
            for ci in range(2):
                for half in range(2):
                    ps = pmed.tile([128, 512], F32, tag="med")
                    for ct in range(8):
                        nc.tensor.matmul(
                            ps,
                            hT[:, ct * 256 + ci * 128:ct * 256 + ci * 128 + 128],
                            vslabs[ct][:, half * 512:(half + 1) * 512],
                            start=(ct == 0), stop=(ct == 7))
                    nc.vector.tensor_copy(
                        out=V_own[:, ci, half * 8:(half + 1) * 8, 0:D],
                        in_=ps.rearrange("p (h d) -> p h d", h=8))
            agv_in = dram.tile([V_ELEMS], F16, tag="agvin")
            agv_out = dram.tile([4 * V_ELEMS], F16, tag="agvout")
            nc.sync.dma_start(
                out=agv_in[:].rearrange("(a p c) -> p a c", a=2, p=128),
                in_=V_own[:].rearrange("p a h d -> p a (h d)"))
            nc.gpsimd.collective_compute(
                "AllGather", ALU.bypass,
                replica_groups=[[0, 1, 2, 3], [4, 5, 6, 7]],
                ins=[agv_in[:].opt()], outs=[agv_out[:].opt()])

            # ---- Q projection (overlaps AG-K) ----
            qslabs = load_slabs(t["wq"], l)
            proj_fm(qslabs, hT, QT)

            # ---- load gathered K/V (one DMA per rank) ----
            for r in range(4):
                kv = agk_out[r * K_ELEMS:(r + 1) * K_ELEMS].rearrange(
                    "(a p c) -> a p c", a=8, p=128)
                nc.scalar.dma_start(
                    out=KT_g[:, :, r, :, :].rearrange("p a s c -> p a (s c)"),
                    in_=kv.rearrange("a p c -> p a c"))
                vv = agv_out[r * V_ELEMS:(r + 1) * V_ELEMS].rearrange(
                    "(a p c) -> a p c", a=2, p=128)
                nc.scalar.dma_start(
                    out=V_g[:, r, :, :, :].rearrange("p s h d -> p s (h d)"),
                    in_=vv.rearrange("a p c -> p a c"))

            # ---- attention ----
            for h in range(H if DBG_ATTN else 0):
                po = (h % 2) * 64
                cth = h // 2
                q0 = QT[po:po + 64, cth * 256:cth * 256 + 128]
                q1 = QT[po:po + 64, cth * 256 + 128:cth * 256 + 256]
                oa = pacc.tile([128, 2, D + 1], F32, tag="oa",
                               name=f"oa_{l}_{h}")
                # qc0: remote chunks {0,1,2} + diag
                ps0 = pmed.tile([128, 512], F32, tag="med")
                for b, j in enumerate((0, 1, 2)):
                    r, s = rs(j)
                    nc.tensor.matmul(ps0[:, b * 128:(b + 1) * 128],
                                     KT_g[po:po + 64, cth, r, s, :], q0,
                                     start=True, stop=True)
                nc.tensor.matmul(ps0[:, 384:512],
                                 KT_own[po:po + 64,
                                        cth * 256:cth * 256 + 128], q0,
                                 start=True, stop=True)
                e0 = epool.tile([128, 512], F16, tag="e0")
                nc.scalar.activation(out=e0, in_=ps0, func=AF.Exp,
                                     scale=ATT_SCALE)
                nc.vector.tensor_mul(out=e0, in0=e0, in1=m0)
                for b, j in enumerate((0, 1, 2)):
                    r, s = rs(j)
                    nc.tensor.matmul(oa[:, 0, :], e0[:, b * 128:(b + 1) * 128],
                                     V_g[:, r, s, h, :],
                                     start=(b == 0), stop=False)
                nc.tensor.matmul(oa[:, 0, :], e0[:, 384:512],
                                 V_own[:, 0, h, :], start=False, stop=True)
                # qc1: remote chunks {0..6} + diag
                ps1 = pbig.tile([128, 1024], F32, tag="big")
                for j in range(7):
                    r, s = rs(j)
                    nc.tensor.matmul(ps1[:, j * 128:(j + 1) * 128],
                                     KT_g[po:po + 64, cth, r, s, :], q1,
                                     start=True, stop=True)
                nc.tensor.matmul(ps1[:, 896:1024],
                                 KT_own[po:po + 64,
                                        cth * 256 + 128:cth * 256 + 256], q1,
                                 start=True, stop=True)
                e1 = epool.tile([128, 1024], F16, tag="e1")
                nc.scalar.activation(out=e1, in_=ps1, func=AF.Exp,
                                     scale=ATT_SCALE)
                nc.vector.tensor_mul(out=e1, in0=e1, in1=m1)
                for j in range(7):
                    r, s = rs(j)
                    nc.tensor.matmul(oa[:, 1, :], e1[:, j * 128:(j + 1) * 128],
                                     V_g[:, r, s, h, :],
                                     start=(j == 0), stop=False)
                nc.tensor.matmul(oa[:, 1, :], e1[:, 896:1024],
                                 V_own[:, 1, h, :], start=False, stop=True)
                # normalize
                for qc in range(2):
                    rec = small.tile([128, 1], F32, tag="rec")
                    nc.vector.reciprocal(out=rec, in_=oa[:, qc, D:D + 1])
                    nc.vector.tensor_scalar_mul(
                        out=o16[:, qc, h * D:(h + 1) * D],
                        in0=oa[:, qc, 0:D], scalar1=rec)

            # ---- o^T via DMA transpose, Wo projection + residual ----
            oview = oT[:].rearrange("p (a two t) -> p a two t", a=8, two=2)
            for ci in range(2 if DBG_WO else 0):
                nc.scalar.dma_start_transpose(out=oview[:, :, ci, :],
                                              in_=o16[:, ci, :])
            woslabs = load_slabs(t["wo"], l)
            bo_sb = small.tile([1, C], F16, tag="borow")
            nc.sync.dma_start(out=bo_sb, in_=t["bo16"][l:l + 1, :])
            x_new = []
            for ci in range(2 if DBG_WO else 0):
                ps = pbig.tile([128, C], F32, tag="big")
                for half in range(2):
                    nc.tensor.matmul(ps[:, half * 512:(half + 1) * 512],
                                     ones1,
                                     bo_sb[:, half * 512:(half + 1) * 512],
                                     start=True, stop=False)
                    for ct in range(8):
                        nc.tensor.matmul(
                            ps[:, half * 512:(half + 1) * 512],
                            oT[:, ct * 256 + ci * 128:ct * 256 + ci * 128 + 128],
                            woslabs[ct][:, half * 512:(half + 1) * 512],
                            start=False, stop=(ct == 7))
                xt = xpool.tile([128, C], F32, tag=f"x{ci}")
                nc.vector.tensor_tensor(out=xt, in0=ps, in1=x_cur[ci],
                                        op=ALU.add)
                x_new.append(xt)
            if x_new:
                x_cur = x_new

            # ---- LN2 + FFN ----
            if not DBG_FFN:
                continue
            hT = hpool.tile([128, 8 * TPC], F16, tag="hT")
            ln_to_fm(x_cur, t["ln2s"], t["ln2b"], l, hT)
            b1col = small.tile([128, FF // 128], F32, tag="b1col")
            nc.sync.dma_start(out=b1col,
                              in_=t["b1f"][l].rearrange("(a b) -> b a", b=128))
            b2_sb = small.tile([1, C], F16, tag="b2row")
            nc.sync.dma_start(out=b2_sb, in_=t["b2_16"][l:l + 1, :])
            ps_f2 = []
            for ci in range(2):
                ps = pbig.tile([128, C], F32, tag="big")
                for half in range(2):
                    nc.tensor.matmul(ps[:, half * 512:(half + 1) * 512],
                                     ones1,
                                     b2_sb[:, half * 512:(half + 1) * 512],
                                     start=True, stop=False)
                ps_f2.append(ps)
            for f in range(FF // 128):
                w1t = w1p.tile([128, 8 * 128], F16, tag="w1t")
                nc.sync.dma_start(
                    out=w1t[:].rearrange("p (a c) -> p a c", a=8),
                    in_=t["w1b"][l, f].rearrange("a p c -> p a c"))
                ps1 = pmed.tile([128, 512], F32, tag="med")
                for ct in range(8):
                    nc.tensor.matmul(ps1[:, 0:256],
                                     w1t[:, ct * 128:(ct + 1) * 128],
                                     hT[:, ct * 256:(ct + 1) * 256],
                                     start=(ct == 0), stop=(ct == 7))
                aT = apool.tile([128, 256], F16, tag="aT")
                nc.scalar.activation(out=aT, in_=ps1[:, 0:256], func=AF.Relu,
                                     bias=b1col[:, f:f + 1], scale=1.0)
                w2t = w2p.tile([128, C], F16, tag="w2t")
                nc.sync.dma_start(out=w2t,
                                  in_=t["w2"][l, f * 128:(f + 1) * 128, :])
                for ci in range(2):
                    for half in range(2):
                        nc.tensor.matmul(
                            ps_f2[ci][:, half * 512:(half + 1) * 512],
                            aT[:, ci * 128:(ci + 1) * 128],
                            w2t[:, half * 512:(half + 1) * 512],
                            start=False, stop=(f == FF // 128 - 1))
            x_new = []
            for ci in range(2):
                xt = xpool.tile([128, C], F32, tag=f"x{ci}")
                nc.vector.tensor_tensor(out=xt, in0=ps_f2[ci], in1=x_cur[ci],
                                        op=ALU.add)
                x_new.append(xt)
            x_cur = x_new

        # ---- final layernorm -> x^T fm fp16 -> AllGather all 8 cores ----
        ln_to_fm(x_cur, t["lnfs"], t["lnfb"], None, xfT)
        agx_in = dram.tile([X_ELEMS], F16, tag="agxin")
        agx_out = dram.tile([NCORE * X_ELEMS], F16, tag="agxout",
                            addr_space="Shared")
        nc.sync.dma_start(
            out=agx_in[:].rearrange("(a p c) -> p a c", a=8, p=128),
            in_=xfT[:].rearrange("p (a c) -> p a c", a=8))
        nc.gpsimd.collective_compute(
            "AllGather", ALU.bypass,
            replica_groups=[list(range(NCORE))],
            ins=[agx_in[:].opt()], outs=[agx_out[:].opt()])
        for r in range(NCORE):
            nc.scalar.dma_start(
                out=XT[:, :, r, :],
                in_=agx_out[r * X_ELEMS:(r + 1) * X_ELEMS]
                .rearrange("(a p c) -> p a c", a=8, p=128))

        # ---- LM head: logits[2048, 4000] = X^T.T @ embT ----
        for vg in range(8 if DBG_LMHEAD else 0):
            evt = evp.tile([128, 8, 500], F16, tag="evt")
            nc.scalar.dma_start(
                out=evt,
                in_=t["embT"].rearrange("(a p) v -> p a v", p=128)
                [:, :, vg * 500:(vg + 1) * 500])
            for tt in range(16):
                ps = pmed.tile([128, 512], F32, tag="med")
                for ct in range(8):
                    nc.tensor.matmul(ps[:, 0:500],
                                     XT[:, ct, tt // 2,
                                        (tt % 2) * 128:(tt % 2) * 128 + 128],
                                     evt[:, ct, :],
                                     start=(ct == 0), stop=(ct == 7))
                lstage = apool.tile([128, 500], F16, tag="lstage")
                nc.vector.tensor_copy(out=lstage, in_=ps[:, 0:500])
                nc.sync.dma_start(
                    out=t["logits"][tt * 128:(tt + 1) * 128,
                                    vg * 500:(vg + 1) * 500],
                    in_=lstage)


_PROG = None


def _get_program():
    global _PROG
    if _PROG is None:
        _PROG = _build_program()
    return _PROG


def _host_inputs(idx, tok_emb, pos_emb, ln1_s, ln1_b, Wq, Wk, Wv, Wo, bo,
                 ln2_s, ln2_b, W1, b1, W2, b2, lnf_s, lnf_b):
    f16 = np.float16
    emb = (tok_emb[idx] + pos_emb[None, :, :]).astype(np.float32)  # (B,T,C)
    wq_t = np.ascontiguousarray(
        np.transpose(Wq, (0, 2, 1, 3)).reshape(L, C, C).astype(f16))
    wk_t = np.ascontiguousarray(
        np.transpose(Wk, (0, 2, 1, 3)).reshape(L, C, C).astype(f16))
    wv_t = np.ascontiguousarray(
        np.transpose(Wv, (0, 2, 1, 3)).reshape(L, C, C).astype(f16))
    w1bk = np.ascontiguousarray(
        W1.reshape(L, 8, 128, 32, 128).transpose(0, 3, 1, 2, 4).astype(f16))
    shared = {
        "wq": wq_t, "wk": wk_t, "wv": wv_t,
        "wo": np.ascontiguousarray(Wo.astype(f16)),
        "w1b": w1bk,
        "w2": np.ascontiguousarray(W2.astype(f16)),
        "bo16": bo.astype(f16), "b2_16": b2.astype(f16),
        "b1f": b1.astype(np.float32),
        "ln1s": ln1_s.astype(np.float32), "ln1b": ln1_b.astype(np.float32),
        "ln2s": ln2_s.astype(np.float32), "ln2b": ln2_b.astype(np.float32),
        "lnfs": lnf_s.astype(np.float32), "lnfb": lnf_b.astype(np.float32),
    }
    embT_full = np.ascontiguousarray(tok_emb.T.astype(f16))  # (C, VOC)
    tril = np.triu(np.ones((128, 128), np.float16))  # [k, q] keep k <= q
    in_maps = []
    for c in range(NCORE):
        b = c // 4
        c4 = c % 4
        g1, g2 = c4, 7 - c4
        x0 = np.concatenate([emb[b, g1 * 128:(g1 + 1) * 128],
                             emb[b, g2 * 128:(g2 + 1) * 128]], axis=0)
        m0 = np.zeros((128, 512), np.float16)
        for j in range(3):
            if j < g1:
                m0[:, j * 128:(j + 1) * 128] = 1.0
        m0[:, 384:512] = tril
        m1 = np.zeros((128, 1024), np.float16)
        for j in range(7):
            if j < g2:
                m1[:, j * 128:(j + 1) * 128] = 1.0
        m1[:, 896:1024] = tril
        m = dict(shared)
        m["x0"] = np.ascontiguousarray(x0.astype(np.float32))
        m["m0d"] = m0
        m["m1d"] = m1
        m["embT"] = np.ascontiguousarray(embT_full[:, c * VS:(c + 1) * VS])
        in_maps.append(m)
    return in_maps


def _assemble(results):
    """results: per-core dicts with 'logits' [2048, VS] fp16 in gathered
    token order (rank-major zigzag chunks). Returns (B, T, VOC) f32."""
    out = np.empty((B, T, VOC), np.float32)
    for c in range(NCORE):
        lg = results[c]["logits"].astype(np.float32)
        vsl = slice(c * VS, (c + 1) * VS)
        for r in range(NCORE):
            rb = r // 4
            r4 = r % 4
            g1, g2 = r4, 7 - r4
            blk = lg[r * 256:(r + 1) * 256]
            out[rb, g1 * 128:(g1 + 1) * 128, vsl] = blk[0:128]
            out[rb, g2 * 128:(g2 + 1) * 128, vsl] = blk[128:256]
    return out


def kernel(**inputs):
    nc = _get_program()
    in_maps = _host_inputs(**inputs)
    res = bass_utils.run_bass_kernel_spmd(
        nc, in_maps, core_ids=list(range(NCORE)))
    return _assemble(res.results)


if __name__ == "__main__":
    import reference as R
    inp = {k: np.asarray(v) for k, v in R.setup_inputs().items()}
    out = kernel(**inp)
    exp = np.asarray(R.reference(**inp))
    err = np.abs(out - exp)
    print("absmax expected:", np.abs(exp).max())
    print("max abs err:", err.max(),
          "rel:", err.max() / np.abs(exp).max())


# revision 18
# speedup vs baseline: 1.2654x; 1.0655x over previous
"""GPT language model forward pass on 8 Trainium2 NeuronCores.

Sequence-parallel transformer: zigzag 128-token chunks (core c of a
4-core batch group owns global chunks {c%4, 7-c%4}), split K / V
AllGathers per layer (each overlapped with the following projections),
superset-static attention (qc0 vs remote chunks {0,1,2}+diag, qc1 vs
{0..6}+diag) with per-core multiplicative masks, one exp per (head,qc),
all transposes on the DMA XBAR, vocab-sharded tied LM head.

Matmul operands are fp16 (fp32 PSUM); residual stream, layernorm stats
and softmax accumulation stay fp32.
"""
import numpy as np

import concourse.bass as bass
import concourse.mybir as mybir
import concourse.tile as tile
from concourse import bacc
from concourse import bass_utils

F32 = mybir.dt.float32
F16 = mybir.dt.float16
F8 = mybir.dt.float8e4
PM_DR = mybir.MatmulPerfMode.DoubleRow
AF = mybir.ActivationFunctionType
ALU = mybir.AluOpType

C = 1024
H = 16
D = 64
L = 8
FF = 4096
VOC = 32000
B = 2
T = 1024
NCORE = 8
VS = VOC // NCORE          # 4000
TPC = 256                  # tokens per core (2 chunks of 128)
K_ELEMS = 8 * 128 * TPC    # staged K^T elements per core (fp16)
V_ELEMS = 2 * 128 * H * (D + 1)
X_ELEMS = C * TPC
ATT_SCALE = 0.125

import os as _os
FP8_FFN = _os.environ.get("FP8_FFN", "0") == "1"
FP8_LM = _os.environ.get("FP8_LM", "0") == "1"
SH = 16.0      # h quant scale (FFN input)
SW1 = 1024.0   # W1 quant scale
SA = 32.0      # hidden activation quant scale
SW2 = 1024.0   # W2 quant scale
SX = 32.0      # final-LN x quant scale
SE = 1024.0    # embT quant scale
DBG_LAYERS = int(_os.environ.get("DBG_LAYERS", L))
DBG_LMHEAD = _os.environ.get("DBG_LMHEAD", "1") == "1"
DBG_ATTN = _os.environ.get("DBG_ATTN", "1") == "1"
DBG_WO = _os.environ.get("DBG_WO", "1") == "1"
DBG_FFN = _os.environ.get("DBG_FFN", "1") == "1"


def _ln_stats(nc, pool, xt):
    """Return (mv, negmu) for token-major xt [128, C]."""
    stats = pool.tile([128, 2, 6], F32, tag="lnstats")
    xv = xt.rearrange("p (a c) -> p a c", a=2)
    for sg in range(2):
        nc.vector.bn_stats(out=stats[:, sg, :], in_=xv[:, sg, :])
    mv = pool.tile([128, 2], F32, tag="lnmv")
    nc.vector.bn_aggr(out=mv, in_=stats)
    negmu = pool.tile([128, 1], F32, tag="lnnegmu")
    nc.vector.tensor_scalar_mul(out=negmu, in0=mv[:, 0:1], scalar1=-1.0)
    return mv, negmu


def _build_program():
    nc = bacc.Bacc("TRN2", target_bir_lowering=False, debug=False,
                   enable_asserts=True, num_devices=NCORE)

    x0 = nc.dram_tensor("x0", [TPC, C], F32, kind="ExternalInput")
    m0d = nc.dram_tensor("m0d", [128, 512], F16, kind="ExternalInput")
    m1d = nc.dram_tensor("m1d", [128, 1024], F16, kind="ExternalInput")
    wq = nc.dram_tensor("wq", [L, C, C], F16, kind="ExternalInput")
    wk = nc.dram_tensor("wk", [L, C, C], F16, kind="ExternalInput")
    wv = nc.dram_tensor("wv", [L, C, C], F16, kind="ExternalInput")
    wo = nc.dram_tensor("wo", [L, C, C], F16, kind="ExternalInput")
    if FP8_FFN:
        w1b = nc.dram_tensor("w1b", [L, FF // 128, 8, 128, 128], F8,
                             kind="ExternalInput")
        w2p = nc.dram_tensor("w2p", [L, FF // 256, 128, 2, C], F8,
                             kind="ExternalInput")
        w2 = None
    else:
        w1b = nc.dram_tensor("w1b", [L, FF // 128, 8, 128, 128], F16,
                             kind="ExternalInput")
        w2 = nc.dram_tensor("w2", [L, FF, C], F16, kind="ExternalInput")
        w2p = None
    bo16 = nc.dram_tensor("bo16", [L, C], F16, kind="ExternalInput")
    b2_16 = nc.dram_tensor("b2_16", [L, C], F16, kind="ExternalInput")
    b1f = nc.dram_tensor("b1f", [L, FF], F32, kind="ExternalInput")
    ln1s = nc.dram_tensor("ln1s", [L, C], F32, kind="ExternalInput")
    ln1b = nc.dram_tensor("ln1b", [L, C], F32, kind="ExternalInput")
    ln2s = nc.dram_tensor("ln2s", [L, C], F32, kind="ExternalInput")
    ln2b = nc.dram_tensor("ln2b", [L, C], F32, kind="ExternalInput")
    lnfs = nc.dram_tensor("lnfs", [C], F32, kind="ExternalInput")
    lnfb = nc.dram_tensor("lnfb", [C], F32, kind="ExternalInput")
    embT = nc.dram_tensor("embT", [C, VS], F8 if FP8_LM else F16,
                          kind="ExternalInput")
    logits = nc.dram_tensor("logits", [B * T, VS], F16, kind="ExternalOutput")

    with tile.TileContext(nc) as tc:
        _body(nc, tc, locals())
    nc.compile()
    return nc


def _body(nc, tc, t):
    from contextlib import ExitStack
    ctx = ExitStack()
    with ctx:
        per = ctx.enter_context(tc.tile_pool(name="per", bufs=1))
        xpool = ctx.enter_context(tc.tile_pool(name="xpool", bufs=2))
        hpool = ctx.enter_context(tc.tile_pool(name="hpool", bufs=2))
        tmp = ctx.enter_context(tc.tile_pool(name="tmp", bufs=2))
        small = ctx.enter_context(tc.tile_pool(name="small", bufs=4))
        wsl = ctx.enter_context(tc.tile_pool(name="wsl", bufs=9))
        w1p = ctx.enter_context(tc.tile_pool(name="w1p", bufs=3))
        wp2 = ctx.enter_context(tc.tile_pool(name="wp2", bufs=3))
        apool = ctx.enter_context(tc.tile_pool(name="apool", bufs=3))
        epool = ctx.enter_context(tc.tile_pool(name="epool", bufs=9))
        evp = ctx.enter_context(tc.tile_pool(name="evp", bufs=2))
        xtp = ctx.enter_context(tc.tile_pool(name="xtp", bufs=1))
        dram = ctx.enter_context(tc.tile_pool(name="dram", bufs=2,
                                              space="DRAM"))
        pbig = ctx.enter_context(tc.tile_pool(name="pbig", bufs=2,
                                              space="PSUM"))
        pmed = ctx.enter_context(tc.tile_pool(name="pmed", bufs=2,
                                              space="PSUM"))
        pacc = ctx.enter_context(tc.tile_pool(name="pacc", bufs=2,
                                              space="PSUM"))

        # ---- persistent tiles ----
        QT = per.tile([128, 8 * TPC], F16)       # col = ct*256 + ci*128 + t
        KT_own = per.tile([128, 8 * TPC], F16)
        V_own = per.tile([128, 2, H, D + 1], F16)
        nc.vector.memset(V_own[:, :, :, D:D + 1], 1.0)
        KT_g = per.tile([128, 8, 4, 2, 128], F16)   # [p, ct, r, sub, tok]
        V_g = per.tile([128, 4, 2, H, D + 1], F16)  # [p, r, sub, h, d]
        m0 = per.tile([128, 512], F16)
        m1 = per.tile([128, 1024], F16)
        o16 = per.tile([128, 2, C], F16)         # attn out, token-major fp16
        oT = per.tile([128, 8 * TPC], F16)
        xfT = per.tile([128, 8 * TPC], F16)
        if FP8_LM:
            xf8 = per.tile([128, 8 * TPC], F8)
        eps_t = per.tile([128, 1], F32)
        nc.vector.memset(eps_t, 1e-5)
        ones1 = per.tile([1, 128], F16)
        nc.vector.memset(ones1, 1.0)

        nc.sync.dma_start(out=m0, in_=t["m0d"][:, :])
        nc.sync.dma_start(out=m1, in_=t["m1d"][:, :])

        x_cur = []
        for ci in range(2):
            xt = xpool.tile([128, C], F32, tag=f"x{ci}")
            nc.sync.dma_start(out=xt, in_=t["x0"][ci * 128:(ci + 1) * 128, :])
            x_cur.append(xt)

        def ln_to_fm(xt_pair, s_dram, b_dram, l, dstT):
            """LayerNorm token-major -> fp16 -> DMA-XBAR transpose into
            feature-major dstT, then per-feature scale/bias."""
            scol = small.tile([128, 8], F32, tag="scol")
            bcol = small.tile([128, 8], F32, tag="bcol")
            src_s = s_dram[l] if l is not None else s_dram
            src_b = b_dram[l] if l is not None else b_dram
            nc.sync.dma_start(out=scol,
                              in_=src_s.rearrange("(a b) -> b a", b=128))
            nc.sync.dma_start(out=bcol,
                              in_=src_b.rearrange("(a b) -> b a", b=128))
            dview = dstT[:].rearrange("p (a two t) -> p a two t", a=8, two=2)
            for ci in range(2):
                xt = xt_pair[ci]
                mv, negmu = _ln_stats(nc, small, xt)
                sq = small.tile([128, 1], F32, tag="lnsq")
                nc.scalar.activation(out=sq, in_=mv[:, 1:2], func=AF.Sqrt,
                                     bias=eps_t, scale=1.0)
                rstd = small.tile([128, 1], F32, tag="lnrstd")
                nc.vector.reciprocal(out=rstd, in_=sq)
                xn16 = tmp.tile([128, C], F16, tag="xn")
                nc.vector.tensor_scalar(out=xn16, in0=xt, scalar1=negmu,
                                        scalar2=rstd,
                                        op0=ALU.add, op1=ALU.mult)
                nc.scalar.dma_start_transpose(out=dview[:, :, ci, :],
                                              in_=xn16[:])
            for ct in range(8):
                sl = dstT[:, ct * 256:(ct + 1) * 256]
                nc.vector.tensor_scalar(out=sl, in0=sl,
                                        scalar1=scol[:, ct:ct + 1],
                                        scalar2=bcol[:, ct:ct + 1],
                                        op0=ALU.mult, op1=ALU.add)

        def load_slabs(w_dram, l):
            slabs = []
            for ct in range(8):
                sl = wsl.tile([128, C], F16, tag="wsl")
                nc.sync.dma_start(out=sl,
                                  in_=w_dram[l, ct * 128:(ct + 1) * 128, :])
                slabs.append(sl)
            return slabs

        def proj_fm(slabs, hT, dstT):
            """dstT[feat, tok] fp16 = W^T @ h^T."""
            for grp in range(8):
                ps = pmed.tile([128, 512], F32, tag="med")
                for ct in range(8):
                    nc.tensor.matmul(
                        ps[:, 0:256], slabs[ct][:, grp * 128:(grp + 1) * 128],
                        hT[:, ct * 256:(ct + 1) * 256],
                        start=(ct == 0), stop=(ct == 7))
                nc.vector.tensor_copy(
                    out=dstT[:, grp * 256:(grp + 1) * 256], in_=ps[:, 0:256])

        # chunk j -> (rank, sub) in the gathered buffers
        def rs(j):
            return (j, 0) if j < 4 else (7 - j, 1)

        HK = K_ELEMS // 2
        HV = V_ELEMS // 2
        for l in range(DBG_LAYERS):
            # ---- LN1 ----
            hT = hpool.tile([128, 8 * TPC], F16, tag="hT")
            ln_to_fm(x_cur, t["ln1s"], t["ln1b"], l, hT)
            if l == 0:
                # warmup collective: absorbs the runtime's first-collective
                # bootstrap concurrently with layer-0 projections. Emitted
                # after LN1 so the XBAR transposes don't serialize on it.
                wseed = small.tile([1, 256], F16, tag="wseed")
                nc.vector.memset(wseed, 0.0)
                warm_in = dram.tile([256], F16, tag="warmin")
                warm_out = dram.tile([NCORE * 256], F16, tag="warmout",
                                     addr_space="Shared")
                nc.sync.dma_start(
                    out=warm_in[:].rearrange("(a b) -> a b", a=1), in_=wseed)
                nc.gpsimd.collective_compute(
                    "AllGather", ALU.bypass,
                    replica_groups=[list(range(NCORE))],
                    ins=[warm_in[:].opt()], outs=[warm_out[:].opt()])

            # ---- K projection -> stage -> AllGather K (two halves) ----
            kslabs = load_slabs(t["wk"], l)
            proj_fm(kslabs, hT, KT_own)
            agk_in = []
            agk_out = []
            for ah in range(2):
                ki = dram.tile([HK], F16, tag=f"agkin{ah}")
                ko = dram.tile([4 * HK], F16, tag=f"agkout{ah}")
                nc.sync.dma_start(
                    out=ki[:].rearrange("(a p c) -> p a c", a=4, p=128),
                    in_=KT_own[:, ah * 1024:(ah + 1) * 1024]
                    .rearrange("p (a c) -> p a c", a=4))
                nc.gpsimd.collective_compute(
                    "AllGather", ALU.bypass,
                    replica_groups=[[0, 1, 2, 3], [4, 5, 6, 7]],
                    ins=[ki[:].opt()], outs=[ko[:].opt()])
                agk_in.append(ki)
                agk_out.append(ko)

            # ---- V projection -> stage -> AllGather V (two halves) ----
            vslabs = load_slabs(t["wv"], l)
            for ci in range(2):
                for half in range(2):
                    ps = pmed.tile([128, 512], F32, tag="med")
                    for ct in range(8):
                        nc.tensor.matmul(
                            ps,
                            hT[:, ct * 256 + ci * 128:ct * 256 + ci * 128 + 128],
                            vslabs[ct][:, half * 512:(half + 1) * 512],
                            start=(ct == 0), stop=(ct == 7))
                    nc.vector.tensor_copy(
                        out=V_own[:, ci, half * 8:(half + 1) * 8, 0:D],
                        in_=ps.rearrange("p (h d) -> p h d", h=8))
            agv_in = []
            agv_out = []
            for ah in range(2):
                vi = dram.tile([HV], F16, tag=f"agvin{ah}")
                vo = dram.tile([4 * HV], F16, tag=f"agvout{ah}")
                nc.sync.dma_start(
                    out=vi[:].rearrange("(a p c) -> p a c", a=2, p=128),
                    in_=V_own[:, :, ah * 8:(ah + 1) * 8, :]
                    .rearrange("p a h d -> p a (h d)"))
                nc.gpsimd.collective_compute(
                    "AllGather", ALU.bypass,
                    replica_groups=[[0, 1, 2, 3], [4, 5, 6, 7]],
                    ins=[vi[:].opt()], outs=[vo[:].opt()])
                agv_in.append(vi)
                agv_out.append(vo)

            # ---- Q projection (overlaps AG-K) ----
            qslabs = load_slabs(t["wq"], l)
            proj_fm(qslabs, hT, QT)

            # ---- load gathered K/V halves (one DMA per rank per half) ----
            for ah in range(2):
                for r in range(4):
                    kv = agk_out[ah][r * HK:(r + 1) * HK].rearrange(
                        "(a p c) -> a p c", a=4, p=128)
                    nc.scalar.dma_start(
                        out=KT_g[:, ah * 4:(ah + 1) * 4, r, :, :]
                        .rearrange("p a s c -> p a (s c)"),
                        in_=kv.rearrange("a p c -> p a c"))
                    vv = agv_out[ah][r * HV:(r + 1) * HV].rearrange(
                        "(a p c) -> a p c", a=2, p=128)
                    nc.scalar.dma_start(
                        out=V_g[:, r, :, ah * 8:(ah + 1) * 8, :]
                        .rearrange("p s h d -> p s (h d)"),
                        in_=vv.rearrange("a p c -> p a c"))

            # ---- attention: two half-passes (heads 0-7, 8-15), each with
            # a QK+exp phase then an eV phase so remote-K/V waits don't
            # block the in-order PE stream ----
            for hp in range(2 if DBG_ATTN else 0):
                es = []
                for hh in range(8):
                    h = hp * 8 + hh
                    po = (h % 2) * 64
                    cth = h // 2
                    q0 = QT[po:po + 64, cth * 256:cth * 256 + 128]
                    q1 = QT[po:po + 64, cth * 256 + 128:cth * 256 + 256]
                    # qc0: remote chunks {0,1,2} + diag
                    ps0 = pmed.tile([128, 512], F32, tag="med")
                    for b, j in enumerate((0, 1, 2)):
                        r, s = rs(j)
                        nc.tensor.matmul(ps0[:, b * 128:(b + 1) * 128],
                                         KT_g[po:po + 64, cth, r, s, :], q0,
                                         start=True, stop=True)
                    nc.tensor.matmul(ps0[:, 384:512],
                                     KT_own[po:po + 64,
                                            cth * 256:cth * 256 + 128], q0,
                                     start=True, stop=True)
                    e0 = epool.tile([128, 512], F16, tag="e0")
                    nc.scalar.activation(out=e0, in_=ps0, func=AF.Exp,
                                         scale=ATT_SCALE)
                    nc.vector.tensor_mul(out=e0, in0=e0, in1=m0)
                    # qc1: remote chunks {0..6} + diag
                    ps1 = pbig.tile([128, 1024], F32, tag="big")
                    for j in range(7):
                        r, s = rs(j)
                        nc.tensor.matmul(ps1[:, j * 128:(j + 1) * 128],
                                         KT_g[po:po + 64, cth, r, s, :], q1,
                                         start=True, stop=True)
                    nc.tensor.matmul(ps1[:, 896:1024],
                                     KT_own[po:po + 64,
                                            cth * 256 + 128:cth * 256 + 256],
                                     q1, start=True, stop=True)
                    e1 = epool.tile([128, 1024], F16, tag="e1")
                    nc.scalar.activation(out=e1, in_=ps1, func=AF.Exp,
                                         scale=ATT_SCALE)
                    nc.vector.tensor_mul(out=e1, in0=e1, in1=m1)
                    es.append((e0, e1))
                for hh in range(8):
                    h = hp * 8 + hh
                    e0, e1 = es[hh]
                    oa = pacc.tile([128, 2, D + 1], F32, tag="oa",
                                   name=f"oa_{l}_{h}")
                    for b, j in enumerate((0, 1, 2)):
                        r, s = rs(j)
                        nc.tensor.matmul(oa[:, 0, :],
                                         e0[:, b * 128:(b + 1) * 128],
                                         V_g[:, r, s, h, :],
                                         start=(b == 0), stop=False)
                    nc.tensor.matmul(oa[:, 0, :], e0[:, 384:512],
                                     V_own[:, 0, h, :], start=False, stop=True)
                    for j in range(7):
                        r, s = rs(j)
                        nc.tensor.matmul(oa[:, 1, :],
                                         e1[:, j * 128:(j + 1) * 128],
                                         V_g[:, r, s, h, :],
                                         start=(j == 0), stop=False)
                    nc.tensor.matmul(oa[:, 1, :], e1[:, 896:1024],
                                     V_own[:, 1, h, :], start=False, stop=True)
                    for qc in range(2):
                        rec = small.tile([128, 1], F32, tag="rec")
                        nc.vector.reciprocal(out=rec, in_=oa[:, qc, D:D + 1])
                        nc.vector.tensor_scalar_mul(
                            out=o16[:, qc, h * D:(h + 1) * D],
                            in0=oa[:, qc, 0:D], scalar1=rec)

            # ---- o^T via DMA transpose, Wo projection + residual ----
            oview = oT[:].rearrange("p (a two t) -> p a two t", a=8, two=2)
            for ci in range(2 if DBG_WO else 0):
                nc.scalar.dma_start_transpose(out=oview[:, :, ci, :],
                                              in_=o16[:, ci, :])
            woslabs = load_slabs(t["wo"], l)
            bo_sb = small.tile([1, C], F16, tag="borow")
            nc.sync.dma_start(out=bo_sb, in_=t["bo16"][l:l + 1, :])
            x_new = []
            for ci in range(2 if DBG_WO else 0):
                ps = pbig.tile([128, C], F32, tag="big")
                for half in range(2):
                    nc.tensor.matmul(ps[:, half * 512:(half + 1) * 512],
                                     ones1,
                                     bo_sb[:, half * 512:(half + 1) * 512],
                                     start=True, stop=False)
                    for ct in range(8):
                        nc.tensor.matmul(
                            ps[:, half * 512:(half + 1) * 512],
                            oT[:, ct * 256 + ci * 128:ct * 256 + ci * 128 + 128],
                            woslabs[ct][:, half * 512:(half + 1) * 512],
                            start=False, stop=(ct == 7))
                xt = xpool.tile([128, C], F32, tag=f"x{ci}")
                nc.vector.tensor_tensor(out=xt, in0=ps, in1=x_cur[ci],
                                        op=ALU.add)
                x_new.append(xt)
            if x_new:
                x_cur = x_new

            # ---- LN2 + FFN ----
            if not DBG_FFN:
                continue
            hT = hpool.tile([128, 8 * TPC], F16, tag="hT")
            ln_to_fm(x_cur, t["ln2s"], t["ln2b"], l, hT)
            b1col = small.tile([128, FF // 128], F32, tag="b1col")
            nc.sync.dma_start(out=b1col,
                              in_=t["b1f"][l].rearrange("(a b) -> b a", b=128))
            b2_sb = small.tile([1, C], F16, tag="b2row")
            nc.sync.dma_start(out=b2_sb, in_=t["b2_16"][l:l + 1, :])
            if FP8_FFN:
                h8 = hpool.tile([128, 8 * TPC], F8, tag="h8")
                nc.vector.tensor_scalar_mul(out=h8, in0=hT, scalar1=SH)
                h8v = h8[:].rearrange("p (a c) -> p a c", a=8)
            ps_f2 = []
            for ci in range(2):
                ps = pbig.tile([128, C], F32, tag="big")
                for half in range(2):
                    nc.tensor.matmul(ps[:, half * 512:(half + 1) * 512],
                                     ones1,
                                     b2_sb[:, half * 512:(half + 1) * 512],
                                     start=True, stop=False)
                ps_f2.append(ps)
            if FP8_FFN:
                for fp in range(FF // 256):
                    aT8 = apool.tile([128, 2, 256], F8, tag="aT8")
                    w2t = wp2.tile([128, 2, C], F8, tag="w2t")
                    nc.sync.dma_start(out=w2t, in_=t["w2p"][l, fp])
                    for fi in range(2):
                        f = fp * 2 + fi
                        w1t = w1p.tile([128, 8 * 128], F8, tag="w1t")
                        nc.sync.dma_start(
                            out=w1t[:].rearrange("p (a c) -> p a c", a=8),
                            in_=t["w1b"][l, f].rearrange("a p c -> p a c"))
                        w1tv = w1t[:].rearrange("p (a c) -> p a c", a=8)
                        ps1 = pmed.tile([128, 512], F32, tag="med")
                        for tp in range(4):
                            nc.tensor.matmul(
                                ps1[:, 0:256],
                                w1tv[:, 2 * tp:2 * tp + 2, :],
                                h8v[:, 2 * tp:2 * tp + 2, :],
                                start=(tp == 0), stop=(tp == 3),
                                perf_mode=PM_DR)
                        nc.scalar.activation(out=aT8[:, fi, :],
                                             in_=ps1[:, 0:256],
                                             func=AF.Relu,
                                             bias=b1col[:, f:f + 1],
                                             scale=SA / (SH * SW1))
                    for ci in range(2):
                        for s in range(4):
                            nc.tensor.matmul(
                                ps_f2[ci][:, s * 256:(s + 1) * 256],
                                aT8[:, :, ci * 128:ci * 128 + 128],
                                w2t[:, :, s * 256:(s + 1) * 256],
                                start=False, stop=(fp == FF // 256 - 1),
                                perf_mode=PM_DR)
            else:
                for f in range(FF // 128):
                    w1t = w1p.tile([128, 8 * 128], F16, tag="w1t")
                    nc.sync.dma_start(
                        out=w1t[:].rearrange("p (a c) -> p a c", a=8),
                        in_=t["w1b"][l, f].rearrange("a p c -> p a c"))
                    ps1 = pmed.tile([128, 512], F32, tag="med")
                    for ct in range(8):
                        nc.tensor.matmul(ps1[:, 0:256],
                                         w1t[:, ct * 128:(ct + 1) * 128],
                                         hT[:, ct * 256:(ct + 1) * 256],
                                         start=(ct == 0), stop=(ct == 7))
                    aT = apool.tile([128, 256], F16, tag="aT")
                    nc.scalar.activation(out=aT, in_=ps1[:, 0:256],
                                         func=AF.Relu,
                                         bias=b1col[:, f:f + 1], scale=1.0)
                    w2t = wp2.tile([128, C], F16, tag="w2t")
                    nc.sync.dma_start(out=w2t,
                                      in_=t["w2"][l, f * 128:(f + 1) * 128, :])
                    for ci in range(2):
                        for half in range(2):
                            nc.tensor.matmul(
                                ps_f2[ci][:, half * 512:(half + 1) * 512],
                                aT[:, ci * 128:(ci + 1) * 128],
                                w2t[:, half * 512:(half + 1) * 512],
                                start=False, stop=(f == FF // 128 - 1))
            x_new = []
            for ci in range(2):
                xt = xpool.tile([128, C], F32, tag=f"x{ci}")
                if FP8_FFN:
                    xs = tmp.tile([128, C], F32, tag="ffnus")
                    nc.vector.tensor_scalar_mul(out=xs, in0=ps_f2[ci],
                                                scalar1=1.0 / (SA * SW2))
                    nc.vector.tensor_tensor(out=xt, in0=xs, in1=x_cur[ci],
                                            op=ALU.add)
                else:
                    nc.vector.tensor_tensor(out=xt, in0=ps_f2[ci],
                                            in1=x_cur[ci], op=ALU.add)
                x_new.append(xt)
            x_cur = x_new

        # ---- final layernorm -> x^T fm fp16 -> AllGather all 8 cores ----
        ln_to_fm(x_cur, t["lnfs"], t["lnfb"], None, xfT)
        agdt = F8 if FP8_LM else F16
        if FP8_LM:
            nc.vector.tensor_scalar_mul(out=xf8, in0=xfT, scalar1=SX)
            agx_src = xf8
        else:
            agx_src = xfT
        agx_in = dram.tile([X_ELEMS], agdt, tag="agxin")
        agx_out = dram.tile([NCORE * X_ELEMS], agdt, tag="agxout",
                            addr_space="Shared")
        nc.sync.dma_start(
            out=agx_in[:].rearrange("(a p c) -> p a c", a=8, p=128),
            in_=agx_src[:].rearrange("p (a c) -> p a c", a=8))
        nc.gpsimd.collective_compute(
            "AllGather", ALU.bypass,
            replica_groups=[list(range(NCORE))],
            ins=[agx_in[:].opt()], outs=[agx_out[:].opt()])
        # ---- LM head: logits[2048, 4000] = X^T.T @ embT ----
        # gathered tokens processed in two rank-halves so the XT buffer
        # is half-sized; embT tiles are re-streamed per half
        for rh in range(2 if DBG_LMHEAD else 0):
            XTh = xtp.tile([128, 8, 4, TPC], agdt, tag="xth")
            for r4 in range(4):
                r = rh * 4 + r4
                nc.scalar.dma_start(
                    out=XTh[:, :, r4, :],
                    in_=agx_out[r * X_ELEMS:(r + 1) * X_ELEMS]
                    .rearrange("(a p c) -> p a c", a=8, p=128))
            for vg in range(8):
                evt = evp.tile([128, 8, 500], agdt, tag="evt")
                nc.scalar.dma_start(
                    out=evt,
                    in_=t["embT"].rearrange("(a p) v -> p a v", p=128)
                    [:, :, vg * 500:(vg + 1) * 500])
                for tt8 in range(8):
                    tt = rh * 8 + tt8
                    ps = pmed.tile([128, 512], F32, tag="med")
                    lstage = apool.tile([128, 500], F16, tag="lstage")
                    if FP8_LM:
                        for n in range(2):
                            for tp in range(4):
                                nc.tensor.matmul(
                                    ps[:, n * 250:(n + 1) * 250],
                                    XTh[:, 2 * tp:2 * tp + 2, tt8 // 2,
                                        (tt8 % 2) * 128:(tt8 % 2) * 128 + 128],
                                    evt[:, 2 * tp:2 * tp + 2,
                                        n * 250:(n + 1) * 250],
                                    start=(tp == 0), stop=(tp == 3),
                                    perf_mode=PM_DR)
                        nc.vector.tensor_scalar_mul(
                            out=lstage, in0=ps[:, 0:500],
                            scalar1=1.0 / (SX * SE))
                    else:
                        for ct in range(8):
                            nc.tensor.matmul(
                                ps[:, 0:500],
                                XTh[:, ct, tt8 // 2,
                                    (tt8 % 2) * 128:(tt8 % 2) * 128 + 128],
                                evt[:, ct, :],
                                start=(ct == 0), stop=(ct == 7))
                        nc.vector.tensor_copy(out=lstage, in_=ps[:, 0:500])
                    nc.sync.dma_start(
                        out=t["logits"][tt * 128:(tt + 1) * 128,
                                        vg * 500:(vg + 1) * 500],
                        in_=lstage)


_PROG = None


def _get_program():
    global _PROG
    if _PROG is None:
        _PROG = _build_program()
    return _PROG


def _host_inputs(idx, tok_emb, pos_emb, ln1_s, ln1_b, Wq, Wk, Wv, Wo, bo,
                 ln2_s, ln2_b, W1, b1, W2, b2, lnf_s, lnf_b):
    import ml_dtypes
    f16 = np.float16
    f8 = ml_dtypes.float8_e4m3
    emb = (tok_emb[idx] + pos_emb[None, :, :]).astype(np.float32)  # (B,T,C)
    wq_t = np.ascontiguousarray(
        np.transpose(Wq, (0, 2, 1, 3)).reshape(L, C, C).astype(f16))
    wk_t = np.ascontiguousarray(
        np.transpose(Wk, (0, 2, 1, 3)).reshape(L, C, C).astype(f16))
    wv_t = np.ascontiguousarray(
        np.transpose(Wv, (0, 2, 1, 3)).reshape(L, C, C).astype(f16))
    w1bk = W1.reshape(L, 8, 128, 32, 128).transpose(0, 3, 1, 2, 4)
    shared = {
        "wq": wq_t, "wk": wk_t, "wv": wv_t,
        "wo": np.ascontiguousarray(Wo.astype(f16)),
        "bo16": bo.astype(f16),
        "ln1s": ln1_s.astype(np.float32), "ln1b": ln1_b.astype(np.float32),
        "ln2s": ln2_s.astype(np.float32), "ln2b": ln2_b.astype(np.float32),
        "lnfs": lnf_s.astype(np.float32), "lnfb": lnf_b.astype(np.float32),
    }
    if FP8_FFN:
        shared["w1b"] = np.ascontiguousarray((w1bk * SW1).astype(f8))
        shared["w2p"] = np.ascontiguousarray(
            (W2.reshape(L, 16, 2, 128, C).transpose(0, 1, 3, 2, 4)
             * SW2).astype(f8))
        shared["b1f"] = (b1 * SA).astype(np.float32)
        shared["b2_16"] = (b2 * (SA * SW2)).astype(f16)
    else:
        shared["w1b"] = np.ascontiguousarray(w1bk.astype(f16))
        shared["w2"] = np.ascontiguousarray(W2.astype(f16))
        shared["b1f"] = b1.astype(np.float32)
        shared["b2_16"] = b2.astype(f16)
    if FP8_LM:
        embT_full = np.ascontiguousarray((tok_emb.T * SE).astype(f8))
    else:
        embT_full = np.ascontiguousarray(tok_emb.T.astype(f16))  # (C, VOC)
    tril = np.triu(np.ones((128, 128), np.float16))  # [k, q] keep k <= q
    in_maps = []
    for c in range(NCORE):
        b = c // 4
        c4 = c % 4
        g1, g2 = c4, 7 - c4
        x0 = np.concatenate([emb[b, g1 * 128:(g1 + 1) * 128],
                             emb[b, g2 * 128:(g2 + 1) * 128]], axis=0)
        m0 = np.zeros((128, 512), np.float16)
        for j in range(3):
            if j < g1:
                m0[:, j * 128:(j + 1) * 128] = 1.0
        m0[:, 384:512] = tril
        m1 = np.zeros((128, 1024), np.float16)
        for j in range(7):
            if j < g2:
                m1[:, j * 128:(j + 1) * 128] = 1.0
        m1[:, 896:1024] = tril
        m = dict(shared)
        m["x0"] = np.ascontiguousarray(x0.astype(np.float32))
        m["m0d"] = m0
        m["m1d"] = m1
        m["embT"] = np.ascontiguousarray(embT_full[:, c * VS:(c + 1) * VS])
        in_maps.append(m)
    return in_maps


def _assemble(results):
    """results: per-core dicts with 'logits' [2048, VS] fp16 in gathered
    token order (rank-major zigzag chunks). Returns (B, T, VOC) f32."""
    out = np.empty((B, T, VOC), np.float32)
    for c in range(NCORE):
        lg = results[c]["logits"].astype(np.float32)
        vsl = slice(c * VS, (c + 1) * VS)
        for r in range(NCORE):
            rb = r // 4
            r4 = r % 4
            g1, g2 = r4, 7 - r4
            blk = lg[r * 256:(r + 1) * 256]
            out[rb, g1 * 128:(g1 + 1) * 128, vsl] = blk[0:128]
            out[rb, g2 * 128:(g2 + 1) * 128, vsl] = blk[128:256]
    return out


def kernel(**inputs):
    nc = _get_program()
    in_maps = _host_inputs(**inputs)
    res = bass_utils.run_bass_kernel_spmd(
        nc, in_maps, core_ids=list(range(NCORE)))
    return _assemble(res.results)


if __name__ == "__main__":
    import reference as R
    inp = {k: np.asarray(v) for k, v in R.setup_inputs().items()}
    out = kernel(**inp)
    exp = np.asarray(R.reference(**inp))
    err = np.abs(out - exp)
    print("absmax expected:", np.abs(exp).max())
    print("max abs err:", err.max(),
          "rel:", err.max() / np.abs(exp).max())
